# revision 1
# baseline (speedup 1.0000x reference)
"""SNN (soft-nearest-neighbor) contrastive loss on 8 Trainium2 NeuronCores.

Math
----
z = concat(x, y) in R^{8192x128};  d_ij = ||z_i - z_j||.
Reference computes, per row i, a softmax-style ratio with the row max
subtracted; the max cancels mathematically, so we compute
    S0_i  = sum_{j != i} exp(-d_ij)          (device + host gather)
    dp_i  = d_{i, pair(i)}                   (device)
    loss  = mean_i( -log( exp(-dp_i)/S0_i + tiny ) )   (host, trivial)

Symmetry halving
----------------
d_ij is symmetric, so each 128-row block R only computes the exp tile for
column blocks R..R+33 (cyclically; 1 self block + 33 forward blocks).
Row-sums over [self + 31 forward blocks] give the forward part of S0;
column-sums (ones-matmul on PE over the bf16 exp tile, blocks R+1..R+33)
are written out and scattered on the host into the mirrored rows. The
antipodal block (distance 32) is computed by BOTH partners; keeping it out
of the row accumulation and counting only the column-sum copy makes every
unordered pair count exactly once.

Device pipeline (one SPMD program, 8 cores, rows sharded 1024/core)
------------------------------------------------------------------
PE: bf16 matmul u^T u (u = bf16(sqrt(2) z)) into PSUM + identity matmul
adding -16384 on the self-diagonal (exp -> 0), + ones-matmul column sums.
DVE: v = (PSUM - ||u_i||^2/2) - ||u_j||^2/2 = -d2 (scalar_tensor_tensor).
ACT: w = Sqrt(-v); E = Exp(-w) (bf16) with fused accum_out row sums.
Sqrt/Exp sit in different ACT table sets, so row subtiles are processed in
batches with all Sqrts before all Exps (explicit same-engine deps).
Each core gets column-ROTATED operands so every tile index is a
compile-time constant: one identical program for all 8 cores.
"""

import os
import sys
from contextlib import ExitStack

import numpy as np

_TRN_REPO = os.environ.get("TRN_RL_REPO", "/opt/trn_rl_repo")
if _TRN_REPO not in sys.path:
    sys.path.insert(0, _TRN_REPO)

import ml_dtypes

BF16 = ml_dtypes.bfloat16

B = 4096
D = 128
N = 2 * B            # 8192 rows of z
NCORES = 8
RPC = N // NCORES    # 1024 rows per core
S = RPC // 128       # 8 row-subtiles per core
CT = 512             # matmul moving tile (one PSUM bank)
SL = 4224            # strip length: self block + 33 forward blocks
ROWL = 4096          # row-accumulated prefix (self + 31 forward blocks)
PT = 1024            # PSUM tile columns (2 banks); strip = 4*1024 + 128 tail
NCH = 10             # colsum chunks of 512 covering rotated cols [0, 5120)
LARGE = 16384.0      # diagonal nuke: d2 -> 16384, d -> 128, exp(-128) -> 0
BATCH = 4            # row-subtiles per ACT table phase

PROFILE = False
LAST_RESULT = None

_cache = {}


def _build_program():
    import concourse.tile as tile
    from bass_rust import add_dep_helper
    from concourse import bacc, mybir

    f32 = mybir.dt.float32
    f16 = mybir.dt.float16
    bf16 = mybir.dt.bfloat16
    AF = mybir.ActivationFunctionType
    OP = mybir.AluOpType

    nc = bacc.Bacc()

    h_ubtr = nc.declare_dram_parameter("ubtr", [128, N], bf16, isOutput=False)
    h_hsqjb = nc.declare_dram_parameter("hsqjb", [128, N], f16, isOutput=False)
    h_dfix = nc.declare_dram_parameter("dfix", [128, CT], bf16, isOutput=False)
    h_ident = nc.declare_dram_parameter("ident", [128, 128], bf16, isOutput=False)
    h_sel4 = nc.declare_dram_parameter("sel4", [128, 16], bf16, isOutput=False)
    h_hsqp = nc.declare_dram_parameter("hsqp", [128, S], f32, isOutput=False)
    h_s0 = nc.declare_dram_parameter("s0", [128, S], f32, isOutput=True)
    h_dp = nc.declare_dram_parameter("dp", [128, S], f32, isOutput=True)
    h_cs = nc.declare_dram_parameter("cs", [NCH, CT], f32, isOutput=True)

    # strip for subtile s covers rotated cols [s*128, s*128 + SL)
    with tile.TileContext(nc) as tc, ExitStack() as ctx:
        const = ctx.enter_context(tc.tile_pool(name="const", bufs=1))
        wpool = ctx.enter_context(tc.tile_pool(name="wbuf", bufs=BATCH))
        vpool = ctx.enter_context(tc.tile_pool(name="vbuf", bufs=4))
        dpool = ctx.enter_context(tc.tile_pool(name="dump", bufs=2))
        pspool = ctx.enter_context(tc.tile_pool(name="ps", bufs=2, space="PSUM"))
        pstail = ctx.enter_context(tc.tile_pool(name="pst", bufs=1, space="PSUM"))
        cspool = ctx.enter_context(tc.tile_pool(name="cps", bufs=1, space="PSUM"))
        misc = ctx.enter_context(tc.tile_pool(name="misc", bufs=2))

        # big operands: strips only touch rotated cols [0, 5120). Fine-grained
        # chunks spread across DMA queues (per-queue bandwidth is ~1/16 of
        # HBM), finest for the first strip's columns, issued first.
        t_ubtr = const.tile([128, 5120], bf16)
        t_hsqjb = const.tile([128, 5120], f16)
        edges = [0, 256, 512, 768, 1024, 1536, 2048, 2560, 3072, 3584,
                 4096, 4608, 5120]
        for a, b in zip(edges[:4], edges[1:4]):
            nc.sync.dma_start(out=t_ubtr[:, a:b], in_=h_ubtr[:, a:b])
            nc.sync.dma_start(out=t_hsqjb[:, a:b], in_=h_hsqjb[:, a:b])

        t_dfix = const.tile([128, CT], bf16)
        nc.sync.dma_start(out=t_dfix[:], in_=h_dfix[:])
        t_ident = const.tile([128, 128], bf16)
        nc.sync.dma_start(out=t_ident[:], in_=h_ident[:])
        t_sel4 = const.tile([128, 16], bf16)
        nc.sync.dma_start(out=t_sel4[:], in_=h_sel4[:])
        t_hsqp = const.tile([128, S], f32)
        nc.sync.dma_start(out=t_hsqp[:], in_=h_hsqp[:])

        for a, b in zip(edges[3:-1], edges[4:]):
            nc.sync.dma_start(out=t_ubtr[:, a:b], in_=h_ubtr[:, a:b])
            nc.sync.dma_start(out=t_hsqjb[:, a:b], in_=h_hsqjb[:, a:b])

        t_zero4 = const.tile([128, 4], bf16)
        nc.vector.memset(t_zero4[:], 0.0)
        t_z512 = const.tile([128, CT], bf16)
        nc.vector.memset(t_z512[:], 0.0)

        # resident colsum accumulators: chunk ch -> tile ch//4, partition ch%4
        cs_acc = []
        for i in range(3):
            cs_i = cspool.tile([4, CT], f32, tag=f"cs{i}", name=f"cs_acc{i}")
            cs_acc.append(cs_i)

        s0_t = const.tile([128, S], f32)
        dp_t = const.tile([128, S], f32)

        # zero the colsum accumulators (matmul with zero weights) and keep
        # the PE busy ~3.5us so the HAM clock gate opens (2.4 GHz) before
        # the first real matmuls arrive
        for rep in range(3):
            for i in range(3):
                nc.tensor.matmul(
                    cs_acc[i][:], t_zero4[:], t_z512[:],
                    start=(rep == 0), stop=False, skip_group_check=True,
                )

        for b0 in range(0, S, BATCH):
            batch = list(range(b0, min(b0 + BATCH, S)))
            ws = {}
            last_sqrt = None
            # ---- Sqrt phase (PE matmuls -> DVE d2 assembly -> ACT sqrt) ----
            for s in batch:
                base = s * 128  # strip start in rotated cols
                w = wpool.tile([128, SL], f32, tag="w")
                ws[s] = w
                # four 1024-col PSUM tiles pair up into two 2048-col v tiles
                # (halves the ACT sqrt instruction count), plus a 128 tail
                for half in range(2):
                    v = vpool.tile([128, 2 * PT], f32, tag="v")
                    for t in (2 * half, 2 * half + 1):
                        c0 = t * PT
                        c1 = c0 + PT
                        ps = pspool.tile([128, PT], f32, tag="ps")
                        for q0 in range(c0, c1, CT):
                            q1 = q0 + CT
                            nc.tensor.matmul(
                                ps[:, q0 - c0:q1 - c0],
                                t_ubtr[:, base:base + 128],
                                t_ubtr[:, base + q0:base + q1],
                                start=True,
                                stop=not (t == 0 and q0 == 0),
                            )
                            if t == 0 and q0 == 0:
                                # self block: nuke the diagonal (cols [0,128))
                                nc.tensor.matmul(
                                    ps[:, 0:CT],
                                    t_ident[:],
                                    t_dfix[:],
                                    start=False,
                                    stop=True,
                                )
                        # v = (P - ||u_i||^2/2) - ||u_j||^2/2 = -d2
                        nc.vector.scalar_tensor_tensor(
                            out=v[:, c0 - 2 * half * PT:c1 - 2 * half * PT],
                            in0=ps[:],
                            scalar=t_hsqp[:, s:s + 1],
                            in1=t_hsqjb[:, base + c0:base + c1],
                            op0=OP.subtract,
                            op1=OP.subtract,
                        )
                    # w = sqrt(-v) = d_ij
                    last_sqrt = nc.scalar.activation(
                        out=w[:, half * 2 * PT:(half + 1) * 2 * PT],
                        in_=v[:],
                        func=AF.Sqrt,
                        scale=-1.0,
                    )
                # antipodal 128-col tail
                pst = pstail.tile([128, 128], f32, tag="pst")
                nc.tensor.matmul(
                    pst[:],
                    t_ubtr[:, base:base + 128],
                    t_ubtr[:, base + ROWL:base + SL],
                    start=True,
                    stop=True,
                )
                vt = misc.tile([128, 128], f32, tag="vt")
                nc.vector.scalar_tensor_tensor(
                    out=vt[:],
                    in0=pst[:],
                    scalar=t_hsqp[:, s:s + 1],
                    in1=t_hsqjb[:, base + ROWL:base + SL],
                    op0=OP.subtract,
                    op1=OP.subtract,
                )
                last_sqrt = nc.scalar.activation(
                    out=w[:, ROWL:SL],
                    in_=vt[:],
                    func=AF.Sqrt,
                    scale=-1.0,
                )
                # ---- pair distance: strip col ROWL + p (tiny DVE) ----
                junk = misc.tile([128, 128], f32, tag="junk")
                nc.vector.tensor_mul(
                    junk[:], w[:, ROWL:ROWL + 128], t_ident[:],
                )
                nc.vector.tensor_reduce(
                    out=dp_t[:, s:s + 1], in_=junk[:],
                    axis=mybir.AxisListType.X, op=OP.add,
                )
            # ---- Exp phase + column sums for the whole batch ----
            for s in batch:
                base = s * 128
                w = ws[s]
                dump = dpool.tile([128, SL], bf16, tag="dump")
                # row-accumulated prefix: self + 31 forward blocks
                e1 = nc.scalar.activation(
                    out=dump[:, 0:ROWL],
                    in_=w[:, 0:ROWL],
                    func=AF.Exp,
                    scale=-1.0,
                    accum_out=s0_t[:, s:s + 1],
                )
                # antipodal block: exp only (counted via column sums)
                e2 = nc.scalar.activation(
                    out=dump[:, ROWL:SL],
                    in_=w[:, ROWL:SL],
                    func=AF.Exp,
                    scale=-1.0,
                )
                if last_sqrt is not None:
                    for e in (e1, e2):
                        add_dep_helper(
                            e.ins, last_sqrt.ins, sync=False,
                            reason="ACT table phase: exp after batch sqrts",
                        )
                # column sums over rotated cols [base+128, base+SL), split at
                # absolute 512 boundaries; chunk j accumulates into
                # cs_acc[j//4] partition j%4 via a one-hot selector lhsT
                lo = base + 128
                hi = base + SL
                j = lo // CT
                while j * CT < hi:
                    a = max(lo, j * CT)
                    b = min(hi, (j + 1) * CT)
                    m = j % 4
                    nc.tensor.matmul(
                        cs_acc[j // 4][:, a - j * CT:b - j * CT],
                        t_sel4[:, 4 * m:4 * m + 4],
                        dump[:, a - base:b - base],
                        start=False,
                        stop=False,
                        skip_group_check=True,
                    )
                    j += 1

        # drain colsum accumulators: PSUM -> SBUF -> DRAM
        for i in range(3):
            sb = misc.tile([4, CT], f32, tag="csdrain")
            nc.vector.tensor_copy(sb[:], cs_acc[i][:])
            nrow = 4 if i < 2 else NCH - 8
            nc.sync.dma_start(out=h_cs[4 * i:4 * i + nrow, :], in_=sb[0:nrow, :])

        nc.sync.dma_start(out=h_s0[:], in_=s0_t[:])
        nc.sync.dma_start(out=h_dp[:], in_=dp_t[:])

    nc.finalize()
    return nc


def get_program():
    if "nc" not in _cache:
        _cache["nc"] = _build_program()
    return _cache["nc"]


def make_in_maps(x, y):
    """Host-side prep: build the per-core (column-rotated) operand arrays."""
    x = np.asarray(x, dtype=np.float32)
    y = np.asarray(y, dtype=np.float32)
    z = np.concatenate([x, y], axis=0)  # [N, D]

    u = (np.float32(np.sqrt(2.0)) * z).astype(BF16)
    uf = u.astype(np.float32)
    hsq = np.float32(0.5) * (uf * uf).sum(axis=1, dtype=np.float32)  # ||u||^2/2

    ubt = np.ascontiguousarray(u.T)  # [D, N] bf16

    dfix = np.zeros((128, CT), dtype=BF16)
    idx = np.arange(128)
    dfix[idx, idx] = BF16(-LARGE)
    ident = np.eye(128, dtype=BF16)
    sel4 = np.zeros((128, 16), dtype=BF16)
    for t in range(4):
        sel4[:, 4 * t + t] = BF16(1.0)

    hsq_f16 = hsq.astype(np.float16)

    in_maps = []
    for c in range(NCORES):
        r0 = c * RPC
        rows = np.arange(r0, r0 + RPC)

        def rot(a):
            return np.ascontiguousarray(np.roll(a, -r0, axis=-1))

        def pcol(vec, sel):  # [RPC] values -> [128, S] per-partition layout
            return np.ascontiguousarray(vec[sel].reshape(S, 128).T)

        in_maps.append(
            {
                "ubtr": rot(ubt),
                "hsqjb": np.ascontiguousarray(
                    np.broadcast_to(np.roll(hsq_f16, -r0)[None, :], (128, N))
                ),
                "dfix": dfix,
                "ident": ident,
                "sel4": sel4,
                "hsqp": pcol(hsq, rows),
            }
        )
    return in_maps


def finish_on_host(results):
    """Gather per-core row sums, column sums, pair distances; final loss."""
    S0 = np.zeros(N, dtype=np.float64)
    DP = np.empty(N, dtype=np.float64)
    for c in range(NCORES):
        r0 = c * RPC
        s0 = np.asarray(results[c]["s0"], dtype=np.float64)  # [128, S]
        dp = np.asarray(results[c]["dp"], dtype=np.float64)
        cs = np.asarray(results[c]["cs"], dtype=np.float64)  # [NCH, CT]
        S0[r0:r0 + RPC] += s0.T.reshape(-1)
        DP[r0:r0 + RPC] = dp.T.reshape(-1)
        # accumulated column sums: rotated col r in [128, 5120) holds the
        # core's total colsum for global row (r0 + r) mod N
        csf = cs.reshape(-1)
        rot = np.arange(128, S * 128 + SL - 128)
        gidx = (r0 + rot) % N
        S0[gidx] += csf[rot]
    tiny = float(np.finfo(np.float32).tiny)
    num = np.exp(-DP)
    loss = -np.log(num / S0 + tiny)
    return np.asarray(loss.mean(), dtype=np.float32)


def kernel(x, y):
    global LAST_RESULT
    from concourse.bass_utils import run_bass_kernel_spmd

    nc = get_program()
    in_maps = make_in_maps(x, y)
    res = run_bass_kernel_spmd(
        nc, in_maps, list(range(NCORES)), trace=PROFILE
    )
    LAST_RESULT = res
    return finish_on_host(res.results)

